# revision 23
# baseline (speedup 1.0000x reference)
"""MoE layer (top-1 routing) on 8 Trainium2 NeuronCores.

Strategy: data-parallel over the batch (16 samples -> 2 per core). Routing
uses only the tiny router tables (16x8 logits), so top-1 expert selection,
the balance loss, and the per-sample expert-weight gather happen on host.
Each core runs a dense per-sample FFN (relu(x @ W1 + b1) @ W2 + b2) for its
2 samples with the gathered expert weights.

Device layout: activations are kept transposed (feature dim on SBUF
partitions) the whole way so both GEMMs contract along the partition dim
with zero on-device transposes:
    GEMM1: H^T[f,l] = sum_d W1[d,f] * X^T[d,l]   (lhsT=W1 natural layout)
    GEMM2: Y^T[m,l] = sum_f W2[f,m] * H^T[f,l]   (lhsT=W2 natural layout)
x/W1/W2 are cast to bf16 on host (PE runs bf16 at 1 row/cycle vs fp32's 4);
accumulation is fp32 in PSUM, biases applied in fp32 on the scalar engine.
"""

import numpy as np
import ml_dtypes

B, L, D, E, DFF = 16, 1024, 512, 8, 2048
N_CORES = 8
SPB = B // N_CORES  # samples per core
KD = D // 128       # contraction chunks over D
KF = DFF // 128     # contraction chunks over DFF
MD = D // 128       # output-row tiles over D
NT = L // 512       # token blocks (moving dim)

_CACHE = {}

# test.py reads exec_time_ns off this after a BASS_TRACE=1 run
LAST_RESULTS = None


def _build_nc():
    import concourse.bass as bass  # noqa: F401  (registers engines)
    import concourse.tile as tile
    from concourse import bacc, mybir

    FP = mybir.dt.float32
    BF = mybir.dt.bfloat16
    AF = mybir.ActivationFunctionType

    nc = bacc.Bacc("TRN2", target_bir_lowering=False, debug=False,
                   num_devices=N_CORES)

    xT = nc.declare_dram_parameter("xT", [SPB, D, L], BF, isOutput=False)
    w1 = nc.declare_dram_parameter("w1", [SPB, D, DFF], BF, isOutput=False)
    b1 = nc.declare_dram_parameter("b1", [SPB, 128, KF], FP, isOutput=False)
    w2 = nc.declare_dram_parameter("w2", [SPB, DFF, D], BF, isOutput=False)
    b2 = nc.declare_dram_parameter("b2", [SPB, 128, MD], FP, isOutput=False)
    outT = nc.declare_dram_parameter("outT", [SPB, D, L], FP, isOutput=True)

    HF = DFF // 2  # f-column half of W1 (prefetch granule)

    with tile.TileContext(nc) as tc:
        with (
            tc.tile_pool(name="wx", bufs=2) as wx,
            tc.tile_pool(name="h", bufs=2) as hp,
            tc.tile_pool(name="o", bufs=4) as op,
            tc.tile_pool(name="ps", bufs=6, space="PSUM") as ps,
            tc.tile_pool(name="psb", bufs=2, space="PSUM") as psb,
        ):
            # Pre-warm the PE HAM clock gate during the DMA prefetch:
            # dummy matmuls on a raw (never-written, hence dep-free)
            # SBUF tensor keep the PE busy from the end of its preamble
            # so the 4096-cycle activity window flips the clock to
            # 2.4 GHz right as the first real operands land.
            scr = nc.alloc_sbuf_tensor("ham_warm_src", [128, 512], BF).ap()
            psw = psb.tile([128, 512], FP, tag="ps2")
            for _ in range(10):
                nc.tensor.matmul(psw[:], scr[:, 0:128], scr[:],
                                 start=True, stop=True)

            prev_gate = None  # gates the next sample's input loads
            for s in range(SPB):
                w1d = w1[s].rearrange("(k p) f -> p k f", p=128)
                xd = xT[s].rearrange("(k p) l -> p k l", p=128)
                w2d = w2[s].rearrange("(k p) d -> p k d", p=128)

                # Each dma_start costs ~600 ns of serial issue time on its
                # engine's sequencer and rides one DMA ring (~150 GB/s);
                # the serial issue order on sync doubles as a bandwidth
                # prioritizer: k-split chunks (contiguous 2-4 KB partition
                # lines) are issued in order of first use, so the critical
                # w1-quarter-0 + x-block-0 window rides 8 rings while later
                # chunks queue up behind it. w2 goes to gpsimd (SWDGE)
                # gated behind early compute so it can't steal bandwidth
                # from the critical window.
                QF = DFF // 4
                w1t = wx.tile([128, KD, DFF], BF)
                xt = wx.tile([128, KD, L], BF)
                b1t = wx.tile([128, KF], FP)
                # (w1 chunk k, x chunk k) pairs in first-use order: the
                # k-th matmul of the first group is gated on just 256 KB
                crit = []
                for k in range(KD):
                    crit.append(nc.sync.dma_start(
                        w1t[:, k, 0:QF], w1d[:, k, 0:QF]))
                    crit.append(nc.sync.dma_start(
                        xt[:, k, 0:512], xd[:, k, 0:512]))
                crit.append(nc.scalar.dma_start(b1t[:], b1[s]))
                if prev_gate is not None:
                    for i in crit:
                        tile.add_dep_helper(
                            i.ins, prev_gate.ins,
                            reason="stagger next sample's loads")
                # remaining loads in PE consumption order (t=0 groups
                # sweep f=0..15 so all w1 first, x token-block 1 last),
                # spread across issue queues so a warm PE does not outrun
                # the ~650 ns/DMA serial issue rate on any one engine
                for k in range(KD):
                    nc.scalar.dma_start(w1t[:, k, QF:HF], w1d[:, k, QF:HF])
                for k in range(KD):
                    nc.gpsimd.dma_start(w1t[:, k, HF:DFF], w1d[:, k, HF:DFF])
                for k in range(KD):
                    nc.sync.dma_start(xt[:, k, 512:L], xd[:, k, 512:L])

                # H^T, bf16, DFF on partitions (16 x [128, L])
                ht = hp.tile([128, KF, L], BF)
                acts1 = {}

                def gemm1_group(f, t):
                    ps1 = ps.tile([128, 512], FP, tag="ps1")
                    for k in range(KD):
                        nc.tensor.matmul(
                            ps1[:],
                            w1t[:, k, f * 128:(f + 1) * 128],
                            xt[:, k, t * 512:(t + 1) * 512],
                            start=(k == 0), stop=(k == KD - 1))
                    acts1[(f, t)] = nc.scalar.activation(
                        ht[:, f, t * 512:(t + 1) * 512], ps1[:],
                        AF.Relu, bias=b1t[:, f:f + 1])

                for f in range(KF):
                    gemm1_group(f, 0)

                # w2 on gpsimd rings, gated until GEMM1 is underway
                w2t = wx.tile([128, KF, D], BF)
                b2t = wx.tile([128, MD], FP)
                w2_loads = [
                    nc.gpsimd.dma_start(w2t[:, k:k + 2, :], w2d[:, k:k + 2, :])
                    for k in range(0, KF, 2)
                ]
                w2_loads.append(nc.scalar.dma_start(b2t[:], b2[s]))
                for i in w2_loads:
                    tile.add_dep_helper(
                        i.ins, acts1[(KF // 2, 0)].ins,
                        reason="defer w2 load until GEMM1 underway")

                for f in range(KF):
                    gemm1_group(f, 1)
                prev_gate = acts1[(0, 1)]

                for m in range(MD):
                    for t in range(NT):
                        last = (s == SPB - 1 and m == MD - 1 and t == NT - 1)
                        # split the very last group so its output flush
                        # overlaps the final matmuls instead of trailing them
                        nsub = 2 if last else 1
                        w = 512 // nsub
                        for u in range(nsub):
                            ps2 = psb.tile([128, 512], FP, tag="ps2")
                            lo = t * 512 + u * w
                            for k in range(KF):
                                nc.tensor.matmul(
                                    ps2[:, 0:w],
                                    w2t[:, k, m * 128:(m + 1) * 128],
                                    ht[:, k, lo:lo + w],
                                    start=(k == 0), stop=(k == KF - 1))
                            ot = op.tile([128, 512], FP, tag="ot")
                            nc.scalar.activation(
                                ot[:, 0:w], ps2[:, 0:w], AF.Identity,
                                bias=b2t[:, m:m + 1])
                            nc.sync.dma_start(
                                outT[s, m * 128:(m + 1) * 128, lo:lo + w],
                                ot[:, 0:w])
    nc.compile()
    return nc


def kernel(x, view_ids, visit_ids, router_view, router_visit, W1, b1, W2, b2):
    global LAST_RESULTS
    from concourse.bass_utils import run_bass_kernel_spmd

    x = np.asarray(x, dtype=np.float32)
    view_ids = np.asarray(view_ids)
    visit_ids = np.asarray(visit_ids)
    router_view = np.asarray(router_view, dtype=np.float32)
    router_visit = np.asarray(router_visit, dtype=np.float32)
    W1 = np.asarray(W1, dtype=np.float32)
    b1 = np.asarray(b1, dtype=np.float32)
    W2 = np.asarray(W2, dtype=np.float32)
    b2 = np.asarray(b2, dtype=np.float32)

    # --- routing + balance loss (B x E = 128 values; host) ---
    logits = router_view[view_ids] + router_visit[visit_ids]  # (B, E) f32
    top1 = logits.argmax(axis=-1)
    lmax = logits.max(axis=-1, keepdims=True)
    ex = np.exp((logits - lmax).astype(np.float64))
    probs = ex / ex.sum(axis=-1, keepdims=True)
    load = probs.mean(axis=0)
    balance_loss = np.float32(-(load * np.log(load)).sum())

    # --- host-side shard prep: gather expert weights, cast, transpose ---
    bf = ml_dtypes.bfloat16
    W1b = W1.astype(bf)
    W2b = W2.astype(bf)
    xT = np.ascontiguousarray(x.transpose(0, 2, 1)).astype(bf)  # (B, D, L)
    w1g = W1b[top1]                                   # (B, D, DFF)
    w2g = W2b[top1]                                   # (B, DFF, D)
    b1g = np.ascontiguousarray(                       # (B, 128, KF)
        b1[top1].reshape(B, KF, 128).transpose(0, 2, 1))
    b2g = np.ascontiguousarray(                       # (B, 128, MD)
        b2[top1].reshape(B, MD, 128).transpose(0, 2, 1))

    if "nc" not in _CACHE:
        _CACHE["nc"] = _build_nc()
    nc = _CACHE["nc"]

    in_maps = []
    for c in range(N_CORES):
        sl = slice(c * SPB, (c + 1) * SPB)
        in_maps.append({
            "xT": np.ascontiguousarray(xT[sl]),
            "w1": np.ascontiguousarray(w1g[sl]),
            "b1": np.ascontiguousarray(b1g[sl]),
            "w2": np.ascontiguousarray(w2g[sl]),
            "b2": np.ascontiguousarray(b2g[sl]),
        })

    res = run_bass_kernel_spmd(nc, in_maps, list(range(N_CORES)))
    LAST_RESULTS = res

    outT = np.concatenate(
        [res.results[c]["outT"] for c in range(N_CORES)], axis=0)  # (B, D, L)
    out = np.ascontiguousarray(outT.transpose(0, 2, 1))            # (B, L, D)
    return out, balance_loss


# revision 24
# speedup vs baseline: 1.0599x; 1.0599x over previous
"""MoE layer (top-1 routing) on 8 Trainium2 NeuronCores.

Strategy: data-parallel over the batch (16 samples -> 2 per core). Routing
uses only the tiny router tables (16x8 logits), so top-1 expert selection,
the balance loss, and the per-sample expert-weight gather happen on host.
Each core runs a dense per-sample FFN (relu(x @ W1 + b1) @ W2 + b2) for its
2 samples with the gathered expert weights.

Device layout: activations are kept transposed (feature dim on SBUF
partitions) the whole way so both GEMMs contract along the partition dim
with zero on-device transposes:
    GEMM1: H^T[f,l] = sum_d W1[d,f] * X^T[d,l]   (lhsT=W1 natural layout)
    GEMM2: Y^T[m,l] = sum_f W2[f,m] * H^T[f,l]   (lhsT=W2 natural layout)
x/W1/W2 are cast to bf16 on host (PE runs bf16 at 1 row/cycle vs fp32's 4);
accumulation is fp32 in PSUM, biases applied in fp32 on the scalar engine.
"""

import numpy as np
import ml_dtypes

B, L, D, E, DFF = 16, 1024, 512, 8, 2048
N_CORES = 8
SPB = B // N_CORES  # samples per core
KD = D // 128       # contraction chunks over D
KF = DFF // 128     # contraction chunks over DFF
MD = D // 128       # output-row tiles over D
NT = L // 512       # token blocks (moving dim)

_CACHE = {}

# test.py reads exec_time_ns off this after a BASS_TRACE=1 run
LAST_RESULTS = None


def _build_nc():
    import concourse.bass as bass  # noqa: F401  (registers engines)
    import concourse.tile as tile
    from concourse import bacc, mybir

    FP = mybir.dt.float32
    BF = mybir.dt.bfloat16
    AF = mybir.ActivationFunctionType

    nc = bacc.Bacc("TRN2", target_bir_lowering=False, debug=False,
                   num_devices=N_CORES)

    xT = nc.declare_dram_parameter("xT", [SPB, D, L], BF, isOutput=False)
    w1 = nc.declare_dram_parameter("w1", [SPB, D, DFF], BF, isOutput=False)
    b1 = nc.declare_dram_parameter("b1", [SPB, 128, KF], FP, isOutput=False)
    w2 = nc.declare_dram_parameter("w2", [SPB, DFF, D], BF, isOutput=False)
    b2 = nc.declare_dram_parameter("b2", [SPB, 128, MD], FP, isOutput=False)
    outT = nc.declare_dram_parameter("outT", [SPB, D, L], FP, isOutput=True)

    HF = DFF // 2  # f-column half of W1 (prefetch granule)

    with tile.TileContext(nc) as tc:
        with (
            tc.tile_pool(name="wx", bufs=2) as wx,
            tc.tile_pool(name="h", bufs=2) as hp,
            tc.tile_pool(name="o", bufs=4) as op,
            tc.tile_pool(name="ps", bufs=6, space="PSUM") as ps,
            tc.tile_pool(name="psb", bufs=2, space="PSUM") as psb,
        ):
            # Pre-warm the PE HAM clock gate during the DMA prefetch:
            # dummy matmuls on a raw (never-written, hence dep-free)
            # SBUF tensor keep the PE busy from the end of its preamble
            # so the 4096-cycle activity window flips the clock to
            # 2.4 GHz right as the first real operands land.
            scr = nc.alloc_sbuf_tensor("ham_warm_src", [128, 512], BF).ap()
            psw = psb.tile([128, 512], FP, tag="ps2")
            for _ in range(10):
                nc.tensor.matmul(psw[:], scr[:, 0:128], scr[:],
                                 start=True, stop=True)

            prev_gate = None  # gates the next sample's input loads
            for s in range(SPB):
                w1d = w1[s].rearrange("(k p) f -> p k f", p=128)
                xd = xT[s].rearrange("(k p) l -> p k l", p=128)
                w2d = w2[s].rearrange("(k p) d -> p k d", p=128)

                # Each dma_start costs ~600 ns of serial issue time on its
                # engine's sequencer and rides one DMA ring (~150 GB/s);
                # the serial issue order on sync doubles as a bandwidth
                # prioritizer: k-split chunks (contiguous 2-4 KB partition
                # lines) are issued in order of first use, so the critical
                # w1-quarter-0 + x-block-0 window rides 8 rings while later
                # chunks queue up behind it. w2 goes to gpsimd (SWDGE)
                # gated behind early compute so it can't steal bandwidth
                # from the critical window.
                QF = DFF // 4
                w1t = wx.tile([128, KD, DFF], BF)
                xt = wx.tile([128, KD, L], BF)
                b1t = wx.tile([128, KF], FP)
                # (w1 chunk k, x chunk k) pairs in first-use order: the
                # k-th matmul of the first group is gated on just 256 KB
                crit = []
                for k in range(KD):
                    crit.append(nc.sync.dma_start(
                        w1t[:, k, 0:QF], w1d[:, k, 0:QF]))
                    crit.append(nc.sync.dma_start(
                        xt[:, k, 0:512], xd[:, k, 0:512]))
                crit.append(nc.scalar.dma_start(b1t[:], b1[s]))
                if prev_gate is not None:
                    for i in crit:
                        tile.add_dep_helper(
                            i.ins, prev_gate.ins,
                            reason="stagger next sample's loads")
                # remaining loads in PE consumption order: all w1 first
                # (t=0 groups sweep f=0..15), x token-block 1 last
                for k in range(KD):
                    nc.sync.dma_start(w1t[:, k, QF:HF], w1d[:, k, QF:HF])
                for k in range(KD):
                    nc.sync.dma_start(w1t[:, k, HF:DFF], w1d[:, k, HF:DFF])
                for k in range(KD):
                    nc.sync.dma_start(xt[:, k, 512:L], xd[:, k, 512:L])

                # H^T, bf16, DFF on partitions (16 x [128, L])
                ht = hp.tile([128, KF, L], BF)
                acts1 = {}

                def gemm1_group(f, t):
                    ps1 = ps.tile([128, 512], FP, tag="ps1")
                    for k in range(KD):
                        nc.tensor.matmul(
                            ps1[:],
                            w1t[:, k, f * 128:(f + 1) * 128],
                            xt[:, k, t * 512:(t + 1) * 512],
                            start=(k == 0), stop=(k == KD - 1))
                    acts1[(f, t)] = nc.scalar.activation(
                        ht[:, f, t * 512:(t + 1) * 512], ps1[:],
                        AF.Relu, bias=b1t[:, f:f + 1])

                for f in range(KF):
                    gemm1_group(f, 0)

                # w2 on gpsimd rings, gated until GEMM1 is underway
                w2t = wx.tile([128, KF, D], BF)
                b2t = wx.tile([128, MD], FP)
                w2_loads = [
                    nc.gpsimd.dma_start(w2t[:, k:k + 2, :], w2d[:, k:k + 2, :])
                    for k in range(0, KF, 2)
                ]
                w2_loads.append(nc.scalar.dma_start(b2t[:], b2[s]))
                for i in w2_loads:
                    tile.add_dep_helper(
                        i.ins, acts1[(KF // 2, 0)].ins,
                        reason="defer w2 load until GEMM1 underway")

                for f in range(KF):
                    gemm1_group(f, 1)
                prev_gate = acts1[(0, 1)]

                for m in range(MD):
                    for t in range(NT):
                        last = (s == SPB - 1 and m == MD - 1 and t == NT - 1)
                        # split the very last group so its output flush
                        # overlaps the final matmuls instead of trailing them
                        nsub = 2 if last else 1
                        w = 512 // nsub
                        for u in range(nsub):
                            ps2 = psb.tile([128, 512], FP, tag="ps2")
                            lo = t * 512 + u * w
                            for k in range(KF):
                                nc.tensor.matmul(
                                    ps2[:, 0:w],
                                    w2t[:, k, m * 128:(m + 1) * 128],
                                    ht[:, k, lo:lo + w],
                                    start=(k == 0), stop=(k == KF - 1))
                            ot = op.tile([128, 512], FP, tag="ot")
                            nc.scalar.activation(
                                ot[:, 0:w], ps2[:, 0:w], AF.Identity,
                                bias=b2t[:, m:m + 1])
                            nc.sync.dma_start(
                                outT[s, m * 128:(m + 1) * 128, lo:lo + w],
                                ot[:, 0:w])
    nc.compile()
    return nc


def kernel(x, view_ids, visit_ids, router_view, router_visit, W1, b1, W2, b2):
    global LAST_RESULTS
    from concourse.bass_utils import run_bass_kernel_spmd

    x = np.asarray(x, dtype=np.float32)
    view_ids = np.asarray(view_ids)
    visit_ids = np.asarray(visit_ids)
    router_view = np.asarray(router_view, dtype=np.float32)
    router_visit = np.asarray(router_visit, dtype=np.float32)
    W1 = np.asarray(W1, dtype=np.float32)
    b1 = np.asarray(b1, dtype=np.float32)
    W2 = np.asarray(W2, dtype=np.float32)
    b2 = np.asarray(b2, dtype=np.float32)

    # --- routing + balance loss (B x E = 128 values; host) ---
    logits = router_view[view_ids] + router_visit[visit_ids]  # (B, E) f32
    top1 = logits.argmax(axis=-1)
    lmax = logits.max(axis=-1, keepdims=True)
    ex = np.exp((logits - lmax).astype(np.float64))
    probs = ex / ex.sum(axis=-1, keepdims=True)
    load = probs.mean(axis=0)
    balance_loss = np.float32(-(load * np.log(load)).sum())

    # --- host-side shard prep: gather expert weights, cast, transpose ---
    bf = ml_dtypes.bfloat16
    W1b = W1.astype(bf)
    W2b = W2.astype(bf)
    xT = np.ascontiguousarray(x.transpose(0, 2, 1)).astype(bf)  # (B, D, L)
    w1g = W1b[top1]                                   # (B, D, DFF)
    w2g = W2b[top1]                                   # (B, DFF, D)
    b1g = np.ascontiguousarray(                       # (B, 128, KF)
        b1[top1].reshape(B, KF, 128).transpose(0, 2, 1))
    b2g = np.ascontiguousarray(                       # (B, 128, MD)
        b2[top1].reshape(B, MD, 128).transpose(0, 2, 1))

    if "nc" not in _CACHE:
        _CACHE["nc"] = _build_nc()
    nc = _CACHE["nc"]

    in_maps = []
    for c in range(N_CORES):
        sl = slice(c * SPB, (c + 1) * SPB)
        in_maps.append({
            "xT": np.ascontiguousarray(xT[sl]),
            "w1": np.ascontiguousarray(w1g[sl]),
            "b1": np.ascontiguousarray(b1g[sl]),
            "w2": np.ascontiguousarray(w2g[sl]),
            "b2": np.ascontiguousarray(b2g[sl]),
        })

    res = run_bass_kernel_spmd(nc, in_maps, list(range(N_CORES)))
    LAST_RESULTS = res

    outT = np.concatenate(
        [res.results[c]["outT"] for c in range(N_CORES)], axis=0)  # (B, D, L)
    out = np.ascontiguousarray(outT.transpose(0, 2, 1))            # (B, L, D)
    return out, balance_loss


# revision 25
# speedup vs baseline: 1.0839x; 1.0227x over previous
"""MoE layer (top-1 routing) on 8 Trainium2 NeuronCores.

Strategy: data-parallel over the batch (16 samples -> 2 per core). Routing
uses only the tiny router tables (16x8 logits), so top-1 expert selection,
the balance loss, and the per-sample expert-weight gather happen on host.
Each core runs a dense per-sample FFN (relu(x @ W1 + b1) @ W2 + b2) for its
2 samples with the gathered expert weights.

Device layout: activations are kept transposed (feature dim on SBUF
partitions) the whole way so both GEMMs contract along the partition dim
with zero on-device transposes:
    GEMM1: H^T[f,l] = sum_d W1[d,f] * X^T[d,l]   (lhsT=W1 natural layout)
    GEMM2: Y^T[m,l] = sum_f W2[f,m] * H^T[f,l]   (lhsT=W2 natural layout)
x/W1/W2 are cast to bf16 on host (PE runs bf16 at 1 row/cycle vs fp32's 4);
accumulation is fp32 in PSUM, biases applied in fp32 on the scalar engine.
"""

import numpy as np
import ml_dtypes

B, L, D, E, DFF = 16, 1024, 512, 8, 2048
N_CORES = 8
SPB = B // N_CORES  # samples per core
KD = D // 128       # contraction chunks over D
KF = DFF // 128     # contraction chunks over DFF
MD = D // 128       # output-row tiles over D
NT = L // 512       # token blocks (moving dim)

_CACHE = {}

# test.py reads exec_time_ns off this after a BASS_TRACE=1 run
LAST_RESULTS = None


def _build_nc():
    import concourse.bass as bass  # noqa: F401  (registers engines)
    import concourse.tile as tile
    from concourse import bacc, mybir

    FP = mybir.dt.float32
    BF = mybir.dt.bfloat16
    AF = mybir.ActivationFunctionType

    nc = bacc.Bacc("TRN2", target_bir_lowering=False, debug=False,
                   num_devices=N_CORES)

    xT = nc.declare_dram_parameter("xT", [SPB, D, L], BF, isOutput=False)
    w1 = nc.declare_dram_parameter("w1", [SPB, D, DFF], BF, isOutput=False)
    b1 = nc.declare_dram_parameter("b1", [SPB, 128, KF], FP, isOutput=False)
    w2 = nc.declare_dram_parameter("w2", [SPB, DFF, D], BF, isOutput=False)
    b2 = nc.declare_dram_parameter("b2", [SPB, 128, MD], FP, isOutput=False)
    outT = nc.declare_dram_parameter("outT", [SPB, D, L], FP, isOutput=True)

    HF = DFF // 2  # f-column half of W1 (prefetch granule)

    with tile.TileContext(nc) as tc:
        with (
            tc.tile_pool(name="wx", bufs=2) as wx,
            tc.tile_pool(name="h", bufs=2) as hp,
            tc.tile_pool(name="o", bufs=4) as op,
            tc.tile_pool(name="ps", bufs=6, space="PSUM") as ps,
            tc.tile_pool(name="psb", bufs=2, space="PSUM") as psb,
        ):
            prev_gate = None  # gates the next sample's input loads
            for s in range(SPB):
                w1d = w1[s].rearrange("(k p) f -> p k f", p=128)
                xd = xT[s].rearrange("(k p) l -> p k l", p=128)
                w2d = w2[s].rearrange("(k p) d -> p k d", p=128)

                # Each dma_start costs ~600 ns of serial issue time on its
                # engine's sequencer and rides one DMA ring (~150 GB/s);
                # the serial issue order on sync doubles as a bandwidth
                # prioritizer: k-split chunks (contiguous 2-4 KB partition
                # lines) are issued in order of first use, so the critical
                # w1-quarter-0 + x-block-0 window rides 8 rings while later
                # chunks queue up behind it. w2 goes to gpsimd (SWDGE)
                # gated behind early compute so it can't steal bandwidth
                # from the critical window.
                QF = DFF // 4
                w1t = wx.tile([128, KD, DFF], BF)
                xt = wx.tile([128, KD, L], BF)
                b1t = wx.tile([128, KF], FP)
                # (w1 chunk k, x chunk k) pairs in first-use order: the
                # k-th matmul of the first group is gated on just 256 KB
                crit = []
                for k in range(KD):
                    crit.append(nc.sync.dma_start(
                        w1t[:, k, 0:QF], w1d[:, k, 0:QF]))
                    crit.append(nc.sync.dma_start(
                        xt[:, k, 0:512], xd[:, k, 0:512]))
                crit.append(nc.scalar.dma_start(b1t[:], b1[s]))
                if prev_gate is not None:
                    for i in crit:
                        tile.add_dep_helper(
                            i.ins, prev_gate.ins,
                            reason="stagger next sample's loads")
                # remaining loads in PE consumption order: all w1 first
                # (t=0 groups sweep f=0..15), x token-block 1 last
                for k in range(KD):
                    nc.sync.dma_start(w1t[:, k, QF:HF], w1d[:, k, QF:HF])
                for k in range(KD):
                    nc.sync.dma_start(w1t[:, k, HF:DFF], w1d[:, k, HF:DFF])
                for k in range(KD):
                    nc.sync.dma_start(xt[:, k, 512:L], xd[:, k, 512:L])

                # H^T, bf16, DFF on partitions (16 x [128, L])
                ht = hp.tile([128, KF, L], BF)
                acts1 = {}

                def gemm1_group(f, t):
                    ps1 = ps.tile([128, 512], FP, tag="ps1")
                    for k in range(KD):
                        nc.tensor.matmul(
                            ps1[:],
                            w1t[:, k, f * 128:(f + 1) * 128],
                            xt[:, k, t * 512:(t + 1) * 512],
                            start=(k == 0), stop=(k == KD - 1))
                    acts1[(f, t)] = nc.scalar.activation(
                        ht[:, f, t * 512:(t + 1) * 512], ps1[:],
                        AF.Relu, bias=b1t[:, f:f + 1])

                for f in range(KF):
                    gemm1_group(f, 0)

                # w2 on gpsimd rings, gated until GEMM1 is underway
                w2t = wx.tile([128, KF, D], BF)
                b2t = wx.tile([128, MD], FP)
                w2_loads = [
                    nc.gpsimd.dma_start(w2t[:, k:k + 2, :], w2d[:, k:k + 2, :])
                    for k in range(0, KF, 2)
                ]
                w2_loads.append(nc.scalar.dma_start(b2t[:], b2[s]))
                for i in w2_loads:
                    tile.add_dep_helper(
                        i.ins, acts1[(KF // 2, 0)].ins,
                        reason="defer w2 load until GEMM1 underway")

                for f in range(KF):
                    gemm1_group(f, 1)
                prev_gate = acts1[(0, 1)]

                for m in range(MD):
                    for t in range(NT):
                        last = (s == SPB - 1 and m == MD - 1 and t == NT - 1)
                        # split the very last group so its output flush
                        # overlaps the final matmuls instead of trailing them
                        nsub = 2 if last else 1
                        w = 512 // nsub
                        for u in range(nsub):
                            ps2 = psb.tile([128, 512], FP, tag="ps2")
                            lo = t * 512 + u * w
                            for k in range(KF):
                                nc.tensor.matmul(
                                    ps2[:, 0:w],
                                    w2t[:, k, m * 128:(m + 1) * 128],
                                    ht[:, k, lo:lo + w],
                                    start=(k == 0), stop=(k == KF - 1))
                            ot = op.tile([128, 512], FP, tag="ot")
                            nc.scalar.activation(
                                ot[:, 0:w], ps2[:, 0:w], AF.Identity,
                                bias=b2t[:, m:m + 1])
                            nc.sync.dma_start(
                                outT[s, m * 128:(m + 1) * 128, lo:lo + w],
                                ot[:, 0:w])
    nc.compile()
    return nc


def kernel(x, view_ids, visit_ids, router_view, router_visit, W1, b1, W2, b2):
    global LAST_RESULTS
    from concourse.bass_utils import run_bass_kernel_spmd

    x = np.asarray(x, dtype=np.float32)
    view_ids = np.asarray(view_ids)
    visit_ids = np.asarray(visit_ids)
    router_view = np.asarray(router_view, dtype=np.float32)
    router_visit = np.asarray(router_visit, dtype=np.float32)
    W1 = np.asarray(W1, dtype=np.float32)
    b1 = np.asarray(b1, dtype=np.float32)
    W2 = np.asarray(W2, dtype=np.float32)
    b2 = np.asarray(b2, dtype=np.float32)

    # --- routing + balance loss (B x E = 128 values; host) ---
    logits = router_view[view_ids] + router_visit[visit_ids]  # (B, E) f32
    top1 = logits.argmax(axis=-1)
    lmax = logits.max(axis=-1, keepdims=True)
    ex = np.exp((logits - lmax).astype(np.float64))
    probs = ex / ex.sum(axis=-1, keepdims=True)
    load = probs.mean(axis=0)
    balance_loss = np.float32(-(load * np.log(load)).sum())

    # --- host-side shard prep: gather expert weights, cast, transpose ---
    bf = ml_dtypes.bfloat16
    W1b = W1.astype(bf)
    W2b = W2.astype(bf)
    xT = np.ascontiguousarray(x.transpose(0, 2, 1)).astype(bf)  # (B, D, L)
    w1g = W1b[top1]                                   # (B, D, DFF)
    w2g = W2b[top1]                                   # (B, DFF, D)
    b1g = np.ascontiguousarray(                       # (B, 128, KF)
        b1[top1].reshape(B, KF, 128).transpose(0, 2, 1))
    b2g = np.ascontiguousarray(                       # (B, 128, MD)
        b2[top1].reshape(B, MD, 128).transpose(0, 2, 1))

    if "nc" not in _CACHE:
        _CACHE["nc"] = _build_nc()
    nc = _CACHE["nc"]

    in_maps = []
    for c in range(N_CORES):
        sl = slice(c * SPB, (c + 1) * SPB)
        in_maps.append({
            "xT": np.ascontiguousarray(xT[sl]),
            "w1": np.ascontiguousarray(w1g[sl]),
            "b1": np.ascontiguousarray(b1g[sl]),
            "w2": np.ascontiguousarray(w2g[sl]),
            "b2": np.ascontiguousarray(b2g[sl]),
        })

    res = run_bass_kernel_spmd(nc, in_maps, list(range(N_CORES)))
    LAST_RESULTS = res

    outT = np.concatenate(
        [res.results[c]["outT"] for c in range(N_CORES)], axis=0)  # (B, D, L)
    out = np.ascontiguousarray(outT.transpose(0, 2, 1))            # (B, L, D)
    return out, balance_loss
